# revision 21
# baseline (speedup 1.0000x reference)
"""Causal single-head attention (B=8, S=2048, D=1024) on 8 TRN2 NeuronCores.

Sharding: data-parallel over batch -- one batch element per core, weights
replicated (no collectives). Each core runs an identical Bass/Tile program,
all matmuls in bf16 with fp32 PSUM accumulation:

  phase 1 (software-pipelined, one stage per engine):
    X loads on the sync HWDGE queue, W loads on the scalar HWDGE queue,
    fp32->bf16 casts on VectorE, 128x128 PE transposes of X (interleaved
    into the projection matmul stream at chain granularity so TensorE never
    sees a long transpose-only stretch and HAM stays at full clock),
    projections on TensorE: Q^T, K^T in [d_out, s] layout; V in [s, d_out].
  phase 2, per 128-row query band (causal blocks only):
    scores [q, k] with the Q^T d-block stationary and K^T moving (N=512, so
    LDWEIGHTS hides under the matmul), diagonal block masked additively,
    exp on ScalarE (scale=1/sqrt(d)) with the softmax denominator taken for
    free via accum_out, P^T via PE transposes, PV matmuls with P^T
    stationary and V moving; the 1/rowsum scale is folded into the
    PSUM->SBUF output copy.
"""

import sys

sys.path.insert(0, "/opt/trn_rl_repo")

import numpy as np

S = 2048
D = 1024
N_CORES = 8
P = 128

_CACHE = {}


def build(s=S, d=D):
    import concourse.bacc as bacc
    import concourse.mybir as mybir
    import concourse.tile as tile

    f32 = mybir.dt.float32
    bf16 = mybir.dt.bfloat16

    SB = s // P          # s-blocks (query bands / V row blocks)
    DB = d // P          # d-blocks
    SCW = min(512, s)    # projection s-chunk width
    SC = s // SCW
    DCW = min(512, d)    # d chunk width (PSUM bank limit)
    DC = d // DCW

    nc = bacc.Bacc("TRN2", target_bir_lowering=False, debug=False)

    xq = nc.dram_tensor("xq", [s, d], f32, kind="ExternalInput").ap()
    xk = nc.dram_tensor("xk", [s, d], f32, kind="ExternalInput").ap()
    xv = nc.dram_tensor("xv", [s, d], f32, kind="ExternalInput").ap()
    wq = nc.dram_tensor("wq", [d, d], f32, kind="ExternalInput").ap()
    wk = nc.dram_tensor("wk", [d, d], f32, kind="ExternalInput").ap()
    wv = nc.dram_tensor("wv", [d, d], f32, kind="ExternalInput").ap()
    out = nc.dram_tensor("out", [s, d], f32, kind="ExternalOutput").ap()

    scale = 1.0 / float(np.sqrt(d))

    with tile.TileContext(nc) as tc:
        with (
            tc.tile_pool(name="consts", bufs=1) as cpool,
            tc.tile_pool(name="qt", bufs=1) as qt_pool,
            tc.tile_pool(name="kt", bufs=1) as kt_pool,
            tc.tile_pool(name="vn", bufs=1) as v_pool,
        ):
            identity = cpool.tile([P, P], bf16, tag="identity")
            from concourse.masks import make_identity
            make_identity(nc, identity)
            # additive causal mask for natural scores [q, k]: keep k <= q
            dmask = cpool.tile([P, P], f32, tag="dmask")
            nc.gpsimd.memset(dmask, 0.0)
            nc.gpsimd.affine_select(
                out=dmask,
                in_=dmask,
                compare_op=mybir.AluOpType.is_ge,
                fill=-1e9,
                base=0,
                # keep where q - k >= 0
                pattern=[[-1, P]],
                channel_multiplier=1,
            )

            qt = [qt_pool.tile([P, s], bf16, tag=f"qt{i}", name=f"qt{i}") for i in range(DB)]
            kt = [kt_pool.tile([P, s], bf16, tag=f"kt{i}", name=f"kt{i}") for i in range(DB)]
            vn = [v_pool.tile([P, d], bf16, tag=f"v{i}", name=f"v{i}") for i in range(SB)]

            # ---------------- phase 1: casts, transposes, projections ------
            with (
                tc.tile_pool(name="stage", bufs=1) as stage,
                tc.tile_pool(name="wpool", bufs=1) as wpool,
                tc.tile_pool(name="xtp", bufs=1) as xt_pool,
                tc.tile_pool(name="ps1", bufs=1, space="PSUM") as ps1,
            ):
                xt = [xt_pool.tile([P, s], bf16, tag=f"xt{i}", name=f"xt{i}") for i in range(DB)]

                BPC = SCW // P  # 128-row blocks per s-chunk
                inputs_spec = [(xq, wq, "q"), (xk, wk, "k"), (xv, wv, "v")]
                wtiles_by_input = {}

                def emit_w(ii):
                    # W loads on the scalar HWDGE queue (otherwise idle in
                    # phase 1), casts on DVE
                    _, w_dram, _ = inputs_spec[ii]
                    eng = nc.scalar
                    wtiles = []
                    for di in range(DB):
                        wf = stage.tile([P, d], f32, tag="wf", bufs=2, name="wf")
                        eng.dma_start(wf, w_dram[di * P : (di + 1) * P, :])
                        wb = wpool.tile([P, d], bf16, tag=f"w{di}", bufs=2, name="wb")
                        nc.vector.tensor_copy(wb, wf)
                        wtiles.append(wb)
                    wtiles_by_input[ii] = wtiles

                def emit_loads(ii, scn):
                    # load (sync queue) + cast bf16 (DVE); the very first
                    # chunk's loads split across both HWDGE queues to cut
                    # startup latency
                    x_dram, _, _ = inputs_spec[ii]
                    first = ii == 0 and scn == 0
                    xbs = []
                    for bi in range(BPC):
                        si = scn * BPC + bi
                        xf = stage.tile([P, d], f32, tag="xf", bufs=4, name="xf")
                        eng = nc.scalar if (first and bi % 2) else nc.sync
                        eng.dma_start(xf, x_dram[si * P : (si + 1) * P, :])
                        xb = stage.tile([P, d], bf16, tag="xb", bufs=6, name="xb")
                        nc.vector.tensor_copy(xb, xf)
                        xbs.append((si, xb))
                    return xbs

                def emit_tgroup(scn, xbs, di):
                    # PE-transpose one d-block of each 128-row tile in the
                    # chunk into xt[di]; copies PSUM->SBUF on DVE
                    for si, xb in xbs:
                        tp = ps1.tile([P, P], bf16, tag="tp", bufs=4, name="tp")
                        nc.tensor.transpose(
                            tp, xb[:, di * P : (di + 1) * P], identity
                        )
                        nc.vector.tensor_copy(
                            xt[di][:, si * P : (si + 1) * P], tp
                        )

                def emit_proj(ii, scn, next_chunk=None):
                    # projection chains for chunk scn, interleaved at chain
                    # granularity with the PE transposes of next_chunk so the
                    # PE never sees a long transpose-only stretch (HAM) and
                    # transposes hide under proj matmuls
                    _, _, kind = inputs_spec[ii]
                    wtiles = wtiles_by_input[ii]
                    chains = []
                    if kind in ("q", "k"):
                        dst = qt if kind == "q" else kt
                        for od in range(DB):
                            def chain(od=od, dst=dst):
                                pp = ps1.tile([P, SCW], f32, tag="proj",
                                              bufs=3, name="pp")
                                for di in range(DB):
                                    nc.tensor.matmul(
                                        pp,
                                        lhsT=wtiles[di][:, od * P : (od + 1) * P],
                                        rhs=xt[di][:, scn * SCW : (scn + 1) * SCW],
                                        start=(di == 0),
                                        stop=(di == DB - 1),
                                    )
                                nc.vector.tensor_copy(
                                    dst[od][:, scn * SCW : (scn + 1) * SCW], pp
                                )
                            chains.append(chain)
                    else:
                        for bi in range(BPC):
                            for dc in range(DC):
                                def chain(bi=bi, dc=dc):
                                    si = scn * BPC + bi
                                    pp = ps1.tile([P, DCW], f32, tag="proj",
                                                  bufs=3, name="pp")
                                    for di in range(DB):
                                        nc.tensor.matmul(
                                            pp,
                                            lhsT=xt[di][:, si * P : (si + 1) * P],
                                            rhs=wtiles[di][:, dc * DCW : (dc + 1) * DCW],
                                            start=(di == 0),
                                            stop=(di == DB - 1),
                                        )
                                    nc.vector.tensor_copy(
                                        vn[si][:, dc * DCW : (dc + 1) * DCW], pp
                                    )
                                chains.append(chain)
                    for ci, chain in enumerate(chains):
                        chain()
                        if next_chunk is not None and ci < DB:
                            nscn, xbs = next_chunk
                            emit_tgroup(nscn, xbs, ci)

                # software pipeline: loads/casts one chunk ahead; transposes
                # of chunk c+1 interleave with projection chains of chunk c
                chunks = [(ii, scn) for ii in range(3) for scn in range(SC)]
                emitted_w = set()

                def ensure_w(ii):
                    if ii < 3 and ii not in emitted_w:
                        emitted_w.add(ii)
                        emit_w(ii)

                ensure_w(0)
                if SC < 3:
                    ensure_w(1)
                    ensure_w(2)
                xbs0 = emit_loads(*chunks[0])
                for di in range(DB):
                    emit_tgroup(chunks[0][1], xbs0, di)
                for idx, (ii, scn) in enumerate(chunks):
                    if scn == max(SC - 2, 0):
                        ensure_w(ii + 1)
                    nxt = None
                    if idx + 1 < len(chunks):
                        nii, nscn = chunks[idx + 1]
                        xbs = emit_loads(nii, nscn)
                        nxt = (nscn, xbs)
                    emit_proj(ii, scn, next_chunk=nxt)

            # ---------------- phase 2: causal attention per q band ---------
            # scores computed NATURAL [q, k] (stationary = Q^T d-block, moving
            # = K^T with N up to 512 so LDWEIGHTS hides under the matmul);
            # row sums fall out of the exp via accum_out; P^T for the PV
            # matmul comes from PE transposes interleaved into the stream.
            with (
                tc.tile_pool(name="pchp", bufs=1) as pch_pool,
                tc.tile_pool(name="ptbp", bufs=1) as ptb_pool,
                tc.tile_pool(name="outp", bufs=1) as out_pool,
                tc.tile_pool(name="ps_sc", bufs=1, space="PSUM") as ps_sc,
                tc.tile_pool(name="ps_pt", bufs=1, space="PSUM") as ps_pt,
                tc.tile_pool(name="ps_pv", bufs=1, space="PSUM") as ps_pv,
            ):
                for qi in range(SB):
                    nkb = qi + 1
                    kspan = nkb * P
                    nch = (kspan + 511) // 512
                    accs = out_pool.tile([P, max(SB * P // 512, 1)], f32,
                                         tag="accs", bufs=2, name="accs")
                    ptbs = []
                    for ch in range(nch):
                        w = min(512, kspan - ch * 512)
                        sc = ps_sc.tile([P, 512], f32, tag="sc", bufs=3,
                                        name="sc")
                        for di in range(DB):
                            nc.tensor.matmul(
                                sc[:, :w],
                                lhsT=qt[di][:, qi * P : (qi + 1) * P],
                                rhs=kt[di][:, ch * 512 : ch * 512 + w],
                                start=(di == 0),
                                stop=(di == DB - 1),
                            )
                        if ch == nch - 1:
                            # diagonal 128-col block is the tail of the band
                            nc.vector.tensor_add(
                                sc[:, w - P : w], sc[:, w - P : w], dmask
                            )
                        pch = pch_pool.tile([P, 512], bf16, tag="pch", bufs=4,
                                            name="pch")
                        nc.scalar.activation(
                            pch[:, :w], sc[:, :w],
                            mybir.ActivationFunctionType.Exp,
                            scale=scale,
                            accum_out=accs[:, ch : ch + 1],
                        )
                        for b in range(w // P):
                            tpp = ps_pt.tile([P, P], bf16, tag="tpp", bufs=3,
                                             name="tpp")
                            nc.tensor.transpose(
                                tpp, pch[:, b * P : (b + 1) * P], identity
                            )
                            ptb = ptb_pool.tile([P, P], bf16, tag="ptb",
                                                bufs=20, name="ptb")
                            nc.vector.tensor_copy(ptb, tpp)
                            ptbs.append(ptb)

                    pvs = [
                        ps_pv.tile([P, DCW], f32, tag=f"pv{i}", bufs=1,
                                   name=f"pv{i}")
                        for i in range(DC)
                    ]
                    for kb in range(nkb):
                        st = kb == 0
                        sp = kb == nkb - 1
                        for i in range(DC):
                            nc.tensor.matmul(
                                pvs[i], lhsT=ptbs[kb],
                                rhs=vn[kb][:, i * DCW : (i + 1) * DCW],
                                start=st, stop=sp,
                            )

                    rowsum = out_pool.tile([P, 1], f32, tag="rowsum", bufs=2,
                                           name="rowsum")
                    nc.vector.reduce_sum(
                        rowsum, accs[:, :nch], axis=mybir.AxisListType.X
                    )
                    recip = out_pool.tile([P, 1], f32, tag="recip", bufs=2)
                    nc.vector.reciprocal(recip, rowsum)
                    ob = out_pool.tile([P, d], f32, tag="ob", bufs=2)
                    for i in range(DC):
                        nc.vector.tensor_scalar_mul(
                            ob[:, i * DCW : (i + 1) * DCW], pvs[i], recip
                        )
                    nc.sync.dma_start(out[qi * P : (qi + 1) * P, :], ob)

    nc.compile()
    return nc


def _get_nc():
    if "nc" not in _CACHE:
        _CACHE["nc"] = build()
    return _CACHE["nc"]


def _run(in_maps, trace=False):
    from concourse.bass_utils import run_bass_kernel_spmd

    nc = _get_nc()
    return run_bass_kernel_spmd(
        nc, in_maps, core_ids=list(range(N_CORES)), trace=trace
    )


def _in_maps(inputs):
    fq = np.ascontiguousarray(np.asarray(inputs["inputs_for_queries"], np.float32))
    fk = np.ascontiguousarray(np.asarray(inputs["inputs_for_keys"], np.float32))
    fv = np.ascontiguousarray(np.asarray(inputs["inputs_for_values"], np.float32))
    WQ = np.ascontiguousarray(np.asarray(inputs["WQ"], np.float32))
    WK = np.ascontiguousarray(np.asarray(inputs["WK"], np.float32))
    WV = np.ascontiguousarray(np.asarray(inputs["WV"], np.float32))
    return [
        {
            "xq": fq[c],
            "xk": fk[c],
            "xv": fv[c],
            "wq": WQ,
            "wk": WK,
            "wv": WV,
        }
        for c in range(N_CORES)
    ]


def kernel(**inputs) -> np.ndarray:
    res = _run(_in_maps(inputs))
    return np.stack([res.results[c]["out"] for c in range(N_CORES)], axis=0)


# revision 23
# speedup vs baseline: 1.1806x; 1.1806x over previous
"""Causal single-head attention (B=8, S=2048, D=1024) on 8 TRN2 NeuronCores.

Sharding: data-parallel over batch -- one batch element per core, weights
replicated (no collectives). Each core runs an identical Bass/Tile program,
all matmuls in bf16 with fp32 PSUM accumulation:

  phase 1 (software-pipelined, one stage per engine):
    X loads on the sync HWDGE queue, W loads on the scalar HWDGE queue,
    fp32->bf16 casts on VectorE, 128x128 PE transposes of X (interleaved
    into the projection matmul stream at chain granularity so TensorE never
    sees a long transpose-only stretch and HAM stays at full clock),
    projections on TensorE: Q^T, K^T in [d_out, s] layout; V in [s, d_out].
  phase 2, per 128-row query band (causal blocks only):
    scores [q, k] with the Q^T d-block stationary and K^T moving (N=512, so
    LDWEIGHTS hides under the matmul), diagonal block masked additively,
    exp on ScalarE (scale=1/sqrt(d)) with the softmax denominator taken for
    free via accum_out, P^T via PE transposes, PV matmuls with P^T
    stationary and V moving; the 1/rowsum scale is folded into the
    PSUM->SBUF output copy.
"""

import sys

sys.path.insert(0, "/opt/trn_rl_repo")

import numpy as np

S = 2048
D = 1024
N_CORES = 8
P = 128

_CACHE = {}


def build(s=S, d=D):
    import concourse.bacc as bacc
    import concourse.mybir as mybir
    import concourse.tile as tile

    f32 = mybir.dt.float32
    bf16 = mybir.dt.bfloat16

    SB = s // P          # s-blocks (query bands / V row blocks)
    DB = d // P          # d-blocks
    SCW = min(512, s)    # projection s-chunk width
    SC = s // SCW
    DCW = min(512, d)    # d chunk width (PSUM bank limit)
    DC = d // DCW

    nc = bacc.Bacc("TRN2", target_bir_lowering=False, debug=False)

    xq = nc.dram_tensor("xq", [s, d], f32, kind="ExternalInput").ap()
    xk = nc.dram_tensor("xk", [s, d], f32, kind="ExternalInput").ap()
    xv = nc.dram_tensor("xv", [s, d], f32, kind="ExternalInput").ap()
    wq = nc.dram_tensor("wq", [d, d], f32, kind="ExternalInput").ap()
    wk = nc.dram_tensor("wk", [d, d], f32, kind="ExternalInput").ap()
    wv = nc.dram_tensor("wv", [d, d], f32, kind="ExternalInput").ap()
    out = nc.dram_tensor("out", [s, d], f32, kind="ExternalOutput").ap()

    scale = 1.0 / float(np.sqrt(d))

    with tile.TileContext(nc) as tc:
        with (
            tc.tile_pool(name="consts", bufs=1) as cpool,
            tc.tile_pool(name="qt", bufs=1) as qt_pool,
            tc.tile_pool(name="kt", bufs=1) as kt_pool,
            tc.tile_pool(name="vn", bufs=1) as v_pool,
        ):
            identity = cpool.tile([P, P], bf16, tag="identity")
            from concourse.masks import make_identity
            make_identity(nc, identity)
            # additive causal mask for natural scores [q, k]: keep k <= q
            dmask = cpool.tile([P, P], f32, tag="dmask")
            nc.gpsimd.memset(dmask, 0.0)
            nc.gpsimd.affine_select(
                out=dmask,
                in_=dmask,
                compare_op=mybir.AluOpType.is_ge,
                fill=-1e9,
                base=0,
                # keep where q - k >= 0
                pattern=[[-1, P]],
                channel_multiplier=1,
            )

            qt = [qt_pool.tile([P, s], bf16, tag=f"qt{i}", name=f"qt{i}") for i in range(DB)]
            kt = [kt_pool.tile([P, s], bf16, tag=f"kt{i}", name=f"kt{i}") for i in range(DB)]
            vn = [v_pool.tile([P, d], bf16, tag=f"v{i}", name=f"v{i}") for i in range(SB)]

            # ---------------- phase 1: casts, transposes, projections ------
            with (
                tc.tile_pool(name="stage", bufs=1) as stage,
                tc.tile_pool(name="wpool", bufs=1) as wpool,
                tc.tile_pool(name="xtp", bufs=1) as xt_pool,
                tc.tile_pool(name="ps1", bufs=1, space="PSUM") as ps1,
            ):
                xt = [xt_pool.tile([P, s], bf16, tag=f"xt{i}", name=f"xt{i}") for i in range(DB)]

                BPC = SCW // P  # 128-row blocks per s-chunk
                inputs_spec = [(xq, wq, "q"), (xk, wk, "k"), (xv, wv, "v")]
                wtiles_by_input = {}

                def emit_w(ii):
                    # W loads on the scalar HWDGE queue (otherwise idle in
                    # phase 1), casts on DVE
                    _, w_dram, _ = inputs_spec[ii]
                    wtiles = []
                    for di in range(DB):
                        wf = stage.tile([P, d], f32, tag="wf", bufs=2, name="wf")
                        nc.scalar.dma_start(wf, w_dram[di * P : (di + 1) * P, :])
                        wb = wpool.tile([P, d], bf16, tag=f"w{di}", bufs=2, name="wb")
                        nc.vector.tensor_copy(wb, wf)
                        wtiles.append(wb)
                    wtiles_by_input[ii] = wtiles

                def emit_loads(ii, scn):
                    # load (sync queue) + cast bf16 (DVE)
                    x_dram, _, _ = inputs_spec[ii]
                    xbs = []
                    for bi in range(BPC):
                        si = scn * BPC + bi
                        xf = stage.tile([P, d], f32, tag="xf", bufs=4, name="xf")
                        nc.sync.dma_start(xf, x_dram[si * P : (si + 1) * P, :])
                        xb = stage.tile([P, d], bf16, tag="xb", bufs=6, name="xb")
                        nc.vector.tensor_copy(xb, xf)
                        xbs.append((si, xb))
                    return xbs

                def emit_tgroup(scn, xbs, di):
                    # PE-transpose one d-block of each 128-row tile in the
                    # chunk into xt[di]; copies PSUM->SBUF on DVE
                    for si, xb in xbs:
                        tp = ps1.tile([P, P], bf16, tag="tp", bufs=4, name="tp")
                        nc.tensor.transpose(
                            tp, xb[:, di * P : (di + 1) * P], identity
                        )
                        nc.vector.tensor_copy(
                            xt[di][:, si * P : (si + 1) * P], tp
                        )

                def emit_proj(ii, scn, next_chunk=None):
                    # projection chains for chunk scn, interleaved at chain
                    # granularity with the PE transposes of next_chunk so the
                    # PE never sees a long transpose-only stretch (HAM) and
                    # transposes hide under proj matmuls
                    _, _, kind = inputs_spec[ii]
                    wtiles = wtiles_by_input[ii]
                    chains = []
                    if kind in ("q", "k"):
                        dst = qt if kind == "q" else kt
                        for od in range(DB):
                            def chain(od=od, dst=dst):
                                pp = ps1.tile([P, SCW], f32, tag="proj",
                                              bufs=3, name="pp")
                                for di in range(DB):
                                    nc.tensor.matmul(
                                        pp,
                                        lhsT=wtiles[di][:, od * P : (od + 1) * P],
                                        rhs=xt[di][:, scn * SCW : (scn + 1) * SCW],
                                        start=(di == 0),
                                        stop=(di == DB - 1),
                                    )
                                nc.vector.tensor_copy(
                                    dst[od][:, scn * SCW : (scn + 1) * SCW], pp
                                )
                            chains.append(chain)
                    else:
                        for bi in range(BPC):
                            for dc in range(DC):
                                def chain(bi=bi, dc=dc):
                                    si = scn * BPC + bi
                                    pp = ps1.tile([P, DCW], f32, tag="proj",
                                                  bufs=3, name="pp")
                                    for di in range(DB):
                                        nc.tensor.matmul(
                                            pp,
                                            lhsT=xt[di][:, si * P : (si + 1) * P],
                                            rhs=wtiles[di][:, dc * DCW : (dc + 1) * DCW],
                                            start=(di == 0),
                                            stop=(di == DB - 1),
                                        )
                                    nc.vector.tensor_copy(
                                        vn[si][:, dc * DCW : (dc + 1) * DCW], pp
                                    )
                                chains.append(chain)
                    for ci, chain in enumerate(chains):
                        chain()
                        if next_chunk is not None and ci < DB:
                            nscn, xbs = next_chunk
                            emit_tgroup(nscn, xbs, ci)

                # software pipeline: loads/casts one chunk ahead; transposes
                # of chunk c+1 interleave with projection chains of chunk c
                chunks = [(ii, scn) for ii in range(3) for scn in range(SC)]
                emitted_w = set()

                def ensure_w(ii):
                    if ii < 3 and ii not in emitted_w:
                        emitted_w.add(ii)
                        emit_w(ii)

                ensure_w(0)
                if SC < 3:
                    ensure_w(1)
                    ensure_w(2)
                xbs0 = emit_loads(*chunks[0])
                for di in range(DB):
                    emit_tgroup(chunks[0][1], xbs0, di)
                for idx, (ii, scn) in enumerate(chunks):
                    if scn == max(SC - 2, 0):
                        ensure_w(ii + 1)
                    nxt = None
                    if idx + 1 < len(chunks):
                        nii, nscn = chunks[idx + 1]
                        xbs = emit_loads(nii, nscn)
                        nxt = (nscn, xbs)
                    emit_proj(ii, scn, next_chunk=nxt)

            # ---------------- phase 2: causal attention per q band ---------
            # scores computed NATURAL [q, k] (stationary = Q^T d-block, moving
            # = K^T with N up to 512 so LDWEIGHTS hides under the matmul);
            # row sums fall out of the exp via accum_out; P^T for the PV
            # matmul comes from PE transposes interleaved into the stream.
            with (
                tc.tile_pool(name="pchp", bufs=1) as pch_pool,
                tc.tile_pool(name="ptbp", bufs=1) as ptb_pool,
                tc.tile_pool(name="outp", bufs=1) as out_pool,
                tc.tile_pool(name="ps_sc", bufs=1, space="PSUM") as ps_sc,
                tc.tile_pool(name="ps_pt", bufs=1, space="PSUM") as ps_pt,
                tc.tile_pool(name="ps_pv", bufs=1, space="PSUM") as ps_pv,
            ):
                for qi in range(SB):
                    nkb = qi + 1
                    kspan = nkb * P
                    nch = (kspan + 511) // 512
                    accs = out_pool.tile([P, max(SB * P // 512, 1)], f32,
                                         tag="accs", bufs=2, name="accs")
                    ptbs = []
                    for ch in range(nch):
                        w = min(512, kspan - ch * 512)
                        sc = ps_sc.tile([P, 512], f32, tag="sc", bufs=3,
                                        name="sc")
                        for di in range(DB):
                            nc.tensor.matmul(
                                sc[:, :w],
                                lhsT=qt[di][:, qi * P : (qi + 1) * P],
                                rhs=kt[di][:, ch * 512 : ch * 512 + w],
                                start=(di == 0),
                                stop=(di == DB - 1),
                            )
                        if ch == nch - 1:
                            # diagonal 128-col block is the tail of the band
                            nc.vector.tensor_add(
                                sc[:, w - P : w], sc[:, w - P : w], dmask
                            )
                        pch = pch_pool.tile([P, 512], bf16, tag="pch", bufs=4,
                                            name="pch")
                        nc.scalar.activation(
                            pch[:, :w], sc[:, :w],
                            mybir.ActivationFunctionType.Exp,
                            scale=scale,
                            accum_out=accs[:, ch : ch + 1],
                        )
                        for b in range(w // P):
                            tpp = ps_pt.tile([P, P], bf16, tag="tpp", bufs=3,
                                             name="tpp")
                            nc.tensor.transpose(
                                tpp, pch[:, b * P : (b + 1) * P], identity
                            )
                            ptb = ptb_pool.tile([P, P], bf16, tag="ptb",
                                                bufs=20, name="ptb")
                            nc.vector.tensor_copy(ptb, tpp)
                            ptbs.append(ptb)

                    pvs = [
                        ps_pv.tile([P, DCW], f32, tag=f"pv{i}", bufs=1,
                                   name=f"pv{i}")
                        for i in range(DC)
                    ]
                    for kb in range(nkb):
                        st = kb == 0
                        sp = kb == nkb - 1
                        for i in range(DC):
                            nc.tensor.matmul(
                                pvs[i], lhsT=ptbs[kb],
                                rhs=vn[kb][:, i * DCW : (i + 1) * DCW],
                                start=st, stop=sp,
                            )

                    rowsum = out_pool.tile([P, 1], f32, tag="rowsum", bufs=2,
                                           name="rowsum")
                    nc.vector.reduce_sum(
                        rowsum, accs[:, :nch], axis=mybir.AxisListType.X
                    )
                    recip = out_pool.tile([P, 1], f32, tag="recip", bufs=2)
                    nc.vector.reciprocal(recip, rowsum)
                    ob = out_pool.tile([P, d], f32, tag="ob", bufs=2)
                    for i in range(DC):
                        nc.vector.tensor_scalar_mul(
                            ob[:, i * DCW : (i + 1) * DCW], pvs[i], recip
                        )
                    nc.sync.dma_start(out[qi * P : (qi + 1) * P, :], ob)

    nc.compile()
    return nc


def _get_nc():
    if "nc" not in _CACHE:
        _CACHE["nc"] = build()
    return _CACHE["nc"]


def _run(in_maps, trace=False):
    from concourse.bass_utils import run_bass_kernel_spmd

    nc = _get_nc()
    return run_bass_kernel_spmd(
        nc, in_maps, core_ids=list(range(N_CORES)), trace=trace
    )


def _in_maps(inputs):
    fq = np.ascontiguousarray(np.asarray(inputs["inputs_for_queries"], np.float32))
    fk = np.ascontiguousarray(np.asarray(inputs["inputs_for_keys"], np.float32))
    fv = np.ascontiguousarray(np.asarray(inputs["inputs_for_values"], np.float32))
    WQ = np.ascontiguousarray(np.asarray(inputs["WQ"], np.float32))
    WK = np.ascontiguousarray(np.asarray(inputs["WK"], np.float32))
    WV = np.ascontiguousarray(np.asarray(inputs["WV"], np.float32))
    return [
        {
            "xq": fq[c],
            "xk": fk[c],
            "xv": fv[c],
            "wq": WQ,
            "wk": WK,
            "wv": WV,
        }
        for c in range(N_CORES)
    ]


def kernel(**inputs) -> np.ndarray:
    res = _run(_in_maps(inputs))
    return np.stack([res.results[c]["out"] for c in range(N_CORES)], axis=0)
